# revision 4
# baseline (speedup 1.0000x reference)
"""Trainium2 Bass kernel for the token-scan problem.

Math: the reference scans T=128 tokens updating (x, rho) and emits
concat([x_T, y_T, v*_T, rho_T.ravel()]).  The x-recurrence depends only on
the (known) token sequence, so the scan unrolls into dense matmuls:

  V    = token_emb[tokens]                 [T, d]
  R    = relu(Dx @ V^T)                    [n, T]
  x_f  = R @ ones  (row sums)
  h    = R^T x_f                           [T]
  a    = vwu^T h  (vwu = U @ (V*w), U = triu-ones; w = decay weights)
  y    = relu(Dy @ ln(a)) * x_f            [n]
  v*   = ln(E @ y)                         [d]
  rho  = vwp^T R^T (vwp = U @ (V*w'))      [d, n]

Sharding: n split across 8 cores (Dx/Dy rows, E/rho columns, x/y slices).
Cross-core comm: ONE AllReduce of [a_partial | mean_partial] (257 floats).
The final E@y partial sums ([d] per core) are reduced + layernormed on
the host during unshard (8x256 values).

Schedule (cost-model driven): the three DMA queues (SP / Act / Pool) load
dxts chunks + consts in parallel so PE can start ~2.7us in; rho is computed
chunk-by-chunk and written out in 8 [128,512] granules on the SP/Act rings
(NOT the Pool ring, which holds the AllReduce) so all pre-collective work —
including the rho writeback — finishes as early as possible.  The tail
after the AllReduce is just: a DRAM->SBUF fetch, the y/v* chain, and one
combined y|vs output DMA.

Precision: big operands ship as bf16 (Dx, rho out) and fp8-e4m3 (Dy x16,
E x64; these only feed the small y/v* output segments; ln is scale
invariant so only Dy's scale needs undoing, folded into the relu).
All matmuls accumulate in fp32 PSUM.
"""

import numpy as np
import ml_dtypes

N, D, V_VOCAB, T = 16384, 256, 32000, 128
DECAY = 0.97
N_CORES = 8
NS = N // N_CORES           # 2048 rows per core
NT = NS // 128              # 16 tiles of 128
YSCL = 2.0 ** -5            # y -> fp8 scale (ln(E@y) is scale invariant)

_cache = {}

# dxts chunk load order: queue assignment below lands chunks in this order
CHUNK_ORDER = [1, 3, 0, 2]


def _build():
    import concourse.bacc as bacc
    import concourse.mybir as mybir
    import concourse.tile as tile

    f32 = mybir.dt.float32
    bf16 = mybir.dt.bfloat16
    f8 = mybir.dt.float8e4
    AF = mybir.ActivationFunctionType
    ALU = mybir.AluOpType

    nc = bacc.Bacc("TRN2", target_bir_lowering=False, debug=False,
                   num_devices=N_CORES)

    # dxts: [128, 4096] bf16, col = c*1024 + h*512 + n''  (n = c*512+n'',
    #       h = d-half) -- Dx^T packed in four 512-n chunks.
    # dyts: [128, 4096] fp8, col = h*2048 + n   (Dy^T * 16)
    # ets:  [128, 4096] fp8, col = i*256 + h*128 + j -> E[h*128+j, i*128+p]*64
    # consts: [128, 770] bf16: vts[0:256] | vwu[256:512] | vwp[512:768]
    i_dxts = nc.dram_tensor("dxts", [128, 2 * NS], bf16, kind="ExternalInput")
    i_dyts = nc.dram_tensor("dyts", [128, 2 * NS], f8, kind="ExternalInput")
    i_ets = nc.dram_tensor("ets", [128, NT * 256], f8, kind="ExternalInput")
    i_consts = nc.dram_tensor("consts", [128, 770], bf16, kind="ExternalInput")

    o_x = nc.dram_tensor("out_x", [128, NT], f32, kind="ExternalOutput")
    o_a = nc.dram_tensor("out_a", [1, 256], bf16, kind="ExternalOutput")
    # combined tail output: cols 0..NT-1 = y (bf16), cols NT..NT+1 = vs
    o_yv = nc.dram_tensor("out_yv", [128, NT + 2], bf16, kind="ExternalOutput")
    o_rho = nc.dram_tensor("out_rho", [256, NS], bf16, kind="ExternalOutput")

    with tile.TileContext(nc) as tc:
        with (
            tc.tile_pool(name="persist", bufs=1) as pp,
            tc.tile_pool(name="rhobuf", bufs=2) as wp,
            tc.tile_pool(name="psBig", bufs=3, space="PSUM") as psBig,
            tc.tile_pool(name="psRc", bufs=4, space="PSUM") as psRc,
            tc.tile_pool(name="psT", bufs=1, space="PSUM") as psT,
            tc.tile_pool(name="dram", bufs=1, space="DRAM") as dram,
        ):
            # ---- bulk loads split over the three DMA rings ----
            consts = pp.tile([128, 770], bf16)
            dxts = pp.tile([128, 2 * NS], bf16)
            dyts = pp.tile([128, 2 * NS], f8)
            ets = pp.tile([128, NT * 256], f8)

            def chunk_cols(c):
                return slice(c * 1024, (c + 1) * 1024)

            # pin the Act function table (Relu+Copy) before the first real
            # Act op; source a memset tile so it doesn't wait on any load.
            pinsrc = pp.tile([1, 1], f32)
            nc.gpsimd.memset(pinsrc[:], 0.0)

            # SP ring: consts, then dxts chunk0
            nc.sync.dma_start(consts[:], i_consts[:])
            nc.sync.dma_start(dxts[:, chunk_cols(0)], i_dxts[:, chunk_cols(0)])
            # Act ring: dxts chunks 1, 2
            nc.scalar.dma_start(dxts[:, chunk_cols(1)], i_dxts[:, chunk_cols(1)])
            nc.scalar.dma_start(dxts[:, chunk_cols(2)], i_dxts[:, chunk_cols(2)])
            # Pool ring: dxts chunk 3, then the tail-only operands
            nc.gpsimd.dma_start(dxts[:, chunk_cols(3)], i_dxts[:, chunk_cols(3)])
            nc.gpsimd.dma_start(dyts[:], i_dyts[:])
            nc.gpsimd.dma_start(ets[:], i_ets[:])

            vts = consts[:, 0:256]
            vwu = consts[:, 256:512]
            vwp = consts[:, 512:768]

            actp = pp.tile([1, 1], f32)
            nc.scalar.activation(actp[:], pinsrc[:], AF.Sqrt,
                                 bias=1.0, scale=0.0)

            # one PSUM bank for all small tiles:
            # cols 0-15 y, 16-17 a-col, 18-19 vs, 20-275 a-row, 276 h, 277 mean
            tail_ps = psT.tile([128, 278], f32, tag="tail")

            # SBUF persistents
            rcols = pp.tile([128, NT * 128], bf16)
            rt = pp.tile([128, NS], bf16)
            rtacc = pp.tile([128, 4], f32)     # scratch accum for rt relus
            xfcol = pp.tile([128, NT], f32)
            xfb = pp.tile([128, NT], bf16)
            rho_sbs = []
            for dc in range(2):
                rho_sb = wp.tile([128, NS], bf16, tag="rho")
                rho_sbs.append(rho_sb)

            def rc_chunk(c, eng):
                # 4 rcols tiles (n on partitions) for chunk c + x_f accum
                for j in range(4):
                    i = c * 4 + j
                    base = c * 1024 + j * 128
                    rc_ps = psRc.tile([128, 128], f32, tag="rc")
                    for h in range(2):
                        nc.tensor.matmul(
                            rc_ps[:],
                            lhsT=dxts[:, base + h * 512: base + h * 512 + 128],
                            rhs=vts[:, h * 128:(h + 1) * 128],
                            start=(h == 0), stop=(h == 1))
                    e = eng[j]
                    dst = rcols[:, i * 128:(i + 1) * 128]
                    if e == 0:
                        nc.vector.tensor_scalar(dst, rc_ps[:], 0.0, None,
                                                ALU.max, ALU.add,
                                                accum_out=xfcol[:, i:i + 1])
                    else:
                        nc.scalar.activation(dst, rc_ps[:], AF.Relu,
                                             accum_out=xfcol[:, i:i + 1])

            def rt_chunk(c, e):
                # rt = relu(V @ Dx^T) (T on partitions) for chunk c
                rt_ps = psBig.tile([128, 512], f32, tag="big")
                for h in range(2):
                    nc.tensor.matmul(
                        rt_ps[:],
                        lhsT=vts[:, h * 128:(h + 1) * 128],
                        rhs=dxts[:, c * 1024 + h * 512: c * 1024 + h * 512 + 512],
                        start=(h == 0), stop=(h == 1))
                dst = rt[:, c * 512:(c + 1) * 512]
                if e == 0:
                    nc.vector.tensor_scalar(dst, rt_ps[:], 0.0, None,
                                            ALU.max, ALU.add,
                                            accum_out=rtacc[:, c:c + 1])
                else:
                    nc.scalar.activation(dst, rt_ps[:], AF.Relu,
                                         accum_out=rtacc[:, c:c + 1])

            # rho writes: 8 granules (dc, c) of [128, 512], alternating
            # SP/Act rings so none sit behind the collective on Pool.
            wring = [nc.sync, nc.scalar]

            def rho_chunk(c, copy_eng, k):
                for dc in range(2):
                    rho_ps = psBig.tile([128, 512], f32, tag="big")
                    nc.tensor.matmul(rho_ps[:],
                                     lhsT=vwp[:, dc * 128:(dc + 1) * 128],
                                     rhs=rt[:, c * 512:(c + 1) * 512],
                                     start=True, stop=True)
                    dst = rho_sbs[dc][:, c * 512:(c + 1) * 512]
                    if copy_eng[dc] == 0:
                        nc.vector.tensor_copy(dst, rho_ps[:])
                    else:
                        nc.scalar.activation(dst, rho_ps[:], AF.Copy)
                    wring[(k + dc) % 2].dma_start(
                        o_rho[dc * 128:(dc + 1) * 128,
                              c * 512:(c + 1) * 512], dst)

            # ---- PE program (issue order == engine order) ----
            c0, c1, c2, c3 = CHUNK_ORDER
            rc_chunk(c0, [0, 0, 0, 0])     # DVE relus early (Act table loads)
            rt_chunk(c0, 0)
            rc_chunk(c1, [0, 0, 0, 0])
            rt_chunk(c1, 0)
            rho_chunk(c0, [0, 1], 0)
            rc_chunk(c2, [1, 0, 1, 0])
            rt_chunk(c2, 1)
            rho_chunk(c1, [0, 1], 0)
            rc_chunk(c3, [1, 0, 1, 0])
            rt_chunk(c3, 1)

            # ---- a-chain: h = R^T x_f ; a_partial = vwu^T h (+ mean) ----
            nc.vector.tensor_copy(xfb[:], xfcol[:])
            for i in range(NT):
                nc.tensor.matmul(tail_ps[:, 276:277],
                                 lhsT=rcols[:, i * 128:(i + 1) * 128],
                                 rhs=xfb[:, i:i + 1],
                                 start=(i == 0), stop=(i == NT - 1))
            h_sb = pp.tile([128, 1], bf16)
            a_sb = pp.tile([1, 256], bf16)
            nc.vector.tensor_copy(h_sb[:], tail_ps[:, 276:277])
            a_ps = tail_ps[0:1, 20:276]
            nc.tensor.matmul(a_ps, lhsT=h_sb[:], rhs=vwu[:],
                             start=True, stop=True)
            nc.tensor.matmul(tail_ps[0:1, 277:278], lhsT=h_sb[:],
                             rhs=consts[:, 769:770],
                             start=True, stop=True)
            m_sb = pp.tile([1, 1], f32)
            nc.vector.tensor_copy(m_sb[:], tail_ps[0:1, 277:278])
            # send the partial already centered: AllReduce is linear, so
            # sum_c (a_c - m_c) == a - mean(a) exactly
            nc.vector.tensor_scalar_sub(a_sb[:], a_ps, m_sb[:])

            a_in = dram.tile([1, 256], bf16)
            a_out = dram.tile([1, 256], bf16)
            nc.sync.dma_start(a_in[:], a_sb[:])

            # remaining rho chunks while a_in flies
            rho_chunk(c2, [0, 1], 0)
            rho_chunk(c3, [0, 1], 0)

            nc.sync.dma_start(o_x[:], xfcol[:])

            nc.gpsimd.collective_compute(
                "AllReduce", ALU.add,
                replica_groups=[list(range(N_CORES))],
                ins=[a_in.opt()], outs=[a_out.opt()],
            )

            # ---- tail: ln(a), y, vs partial ----
            afull = pp.tile([1, 256], bf16)
            nc.sync.dma_start(afull[:], a_out[:])
            nc.scalar.dma_start(o_a[:], afull[:])

            # transpose (a-m) to columns, folding *2^-10: 2 K=1 matmuls
            for h in range(2):
                nc.tensor.matmul(tail_ps[:, 16 + h:17 + h],
                                 lhsT=afull[0:1, h * 128:(h + 1) * 128],
                                 rhs=consts[0:1, 768:769],
                                 start=True, stop=True)
            aln = pp.tile([128, 2], f8)
            nc.vector.tensor_copy(aln[:], tail_ps[:, 16:18])

            # y = relu(Dy @ aln)/16 * x_f
            for i in range(NT):
                for h in range(2):
                    nc.tensor.matmul(
                        tail_ps[:, i:i + 1],
                        lhsT=dyts[:, h * NS + i * 128: h * NS + (i + 1) * 128],
                        rhs=aln[:, h:h + 1],
                        start=(h == 0), stop=(h == 1))
            ycrs = pp.tile([128, NT], f32)
            y_f8 = pp.tile([128, NT], f8)
            yv_sb = pp.tile([128, NT + 2], bf16)
            nc.scalar.activation(ycrs[:], tail_ps[:, 0:NT], AF.Relu,
                                 scale=2.0 ** -8)
            nc.vector.tensor_mul(y_f8[:], ycrs[:], xfcol[:])
            ycr = pp.tile([128, NT], f32)
            nc.scalar.activation(ycr[:], tail_ps[:, 0:NT], AF.Relu)
            nc.vector.tensor_mul(yv_sb[:, 0:NT], ycr[:], xfcol[:])

            # vs partial = E @ y  (E shipped x64; host ln is scale-inv)
            for h in range(2):
                for i in range(NT):
                    nc.tensor.matmul(
                        tail_ps[:, 18 + h:19 + h],
                        lhsT=ets[:, i * 256 + h * 128: i * 256 + (h + 1) * 128],
                        rhs=y_f8[:, i:i + 1],
                        start=(i == 0), stop=(i == NT - 1))
            nc.vector.tensor_copy(yv_sb[:, NT:NT + 2], tail_ps[:, 18:20])
            nc.sync.dma_start(o_yv[:], yv_sb[:])

    nc.finalize()
    return nc


def _host_prep(E, Dx, Dy, token_emb, tokens):
    bf = ml_dtypes.bfloat16
    f8 = ml_dtypes.float8_e4m3fn
    E = np.asarray(E, dtype=np.float32)
    Dx = np.asarray(Dx, dtype=np.float32)
    Dy = np.asarray(Dy, dtype=np.float32)
    token_emb = np.asarray(token_emb, dtype=np.float32)
    tokens = np.asarray(tokens).astype(np.int64)

    v = np.ascontiguousarray(token_emb[tokens])          # [T, d]
    vts = np.concatenate([v[:, :128].T, v[:, 128:].T], axis=1)  # [128, 256]
    j = np.arange(T)
    w = (DECAY ** ((T - 1) - j)).astype(np.float32)
    w[T - 1] = 0.0
    wp = (DECAY ** (T - j)).astype(np.float32)
    u = np.triu(np.ones((T, T), dtype=np.float32))
    vwu = u @ (v * w[:, None])                           # [T, d]
    vwp = u @ (v * wp[:, None])
    c9 = np.zeros((128, 1), np.float32)
    c9[0, 0] = 2.0 ** -10
    w256 = (vwu.sum(axis=1, keepdims=True) / 256.0).astype(np.float32)
    consts = np.ascontiguousarray(
        np.concatenate([vts, vwu, vwp, c9, w256], axis=1)).astype(bf)

    in_maps = []
    for k in range(N_CORES):
        sl = slice(k * NS, (k + 1) * NS)
        dx_s = Dx[sl]                                    # [NS, 256]
        dy_s = Dy[sl]
        e_s = E[:, sl]                                   # [256, NS]
        dxts = np.empty((128, 2 * NS), np.float32)
        for c in range(4):
            nsl = slice(c * 512, (c + 1) * 512)
            dxts[:, c * 1024: c * 1024 + 512] = dx_s[nsl, :128].T
            dxts[:, c * 1024 + 512: (c + 1) * 1024] = dx_s[nsl, 128:].T
        dyts = np.concatenate([dy_s[:, :128].T, dy_s[:, 128:].T],
                              axis=1) * 16.0
        ets = np.concatenate(
            [e_s[:, i * 128:(i + 1) * 128].T for i in range(NT)],
            axis=1) * 64.0
        in_maps.append({
            "dxts": np.ascontiguousarray(dxts).astype(bf),
            "dyts": np.ascontiguousarray(dyts).astype(f8),
            "ets": np.ascontiguousarray(ets).astype(f8),
            "consts": consts,
        })
    return in_maps


def _ln_host(z, eps=1e-6):
    m = z.mean()
    s = z.std(ddof=1)
    return (z - m) / (s + eps)


def kernel(E, Dx, Dy, token_emb, tokens, _trace=False):
    from concourse.bass_utils import run_bass_kernel_spmd

    key = "nc"
    if key not in _cache:
        _cache[key] = _build()
    nc = _cache[key]

    in_maps = _host_prep(E, Dx, Dy, token_emb, tokens)
    res = run_bass_kernel_spmd(nc, in_maps, core_ids=list(range(N_CORES)),
                               trace=_trace)
    _cache["last_result"] = res

    r = res.results
    x_full = np.concatenate(
        [np.asarray(r[k]["out_x"], np.float32).T.ravel()
         for k in range(N_CORES)])
    a_full = np.asarray(r[0]["out_a"], np.float32).ravel()
    yfac = 64.0 / (a_full.std(ddof=1) + 1e-6)
    y_full = np.concatenate(
        [np.asarray(r[k]["out_yv"]).astype(np.float32)[:, 0:NT].T.ravel()
         * yfac for k in range(N_CORES)])
    vs_raw = np.zeros(256, np.float64)
    for k in range(N_CORES):
        vs_raw += np.asarray(r[k]["out_yv"]).astype(np.float32)[:, NT:].T.ravel()
    vs = _ln_host(vs_raw.astype(np.float32))
    rho = np.concatenate(
        [np.asarray(r[k]["out_rho"]).astype(np.float32)
         for k in range(N_CORES)], axis=1)
    return np.concatenate([x_full, y_full, vs, rho.ravel()]).astype(np.float32)


# revision 7
# speedup vs baseline: 1.1084x; 1.1084x over previous
"""Trainium2 Bass kernel for the token-scan problem.

Math: the reference scans T=128 tokens updating (x, rho) and emits
concat([x_T, y_T, v*_T, rho_T.ravel()]).  The x-recurrence depends only on
the (known) token sequence, so the scan unrolls into dense matmuls:

  V    = token_emb[tokens]                 [T, d]
  R    = relu(Dx @ V^T)                    [n, T]
  x_f  = R @ ones  (row sums)
  h    = R^T x_f                           [T]
  a    = vwu^T h  (vwu = U @ (V*w), U = triu-ones; w = decay weights)
  y    = relu(Dy @ ln(a)) * x_f            [n]
  v*   = ln(E @ y)                         [d]
  rho  = vwp^T R^T (vwp = U @ (V*w'))      [d, n]

Sharding: n split across 8 cores (Dx/Dy rows, E/rho columns, x/y slices).
Cross-core comm: ONE AllReduce of the centered a-partial, shipped in
column layout [128, 2] (the centering  a - mean(a)*ones  is folded into
vwu on the host: vwu'' = (vwu - rowmean) * 2^-10, so the device-side
partial comes out of the PE pre-centered and pre-scaled, in columns --
no transpose matmuls and no mean matmul needed).  The final E@y partial
sums ([d] per core) are reduced + layernormed on the host during unshard.

Schedule (cost-model driven): loads go on the SP and Pool rings only (the
Act ring opens with its 1283ns activation-table load, so it gets no
loads); rho is computed chunk-by-chunk and written out in 8 [128,512]
granules spread over all three rings, ordered so the last granule's
copy+DMA chain starts as early as possible.  Everything -- including the
rho writeback -- completes before the AllReduce; the tail after it is
just: a_out fetch, y/v* chain, one combined y|vs output DMA.

Precision: big operands ship as bf16 (Dx, rho out) and fp8-e4m3 (Dy x16,
E x64; these only feed the small y/v* output segments; ln is scale
invariant so only Dy's scale needs undoing, folded into the relu).
All matmuls accumulate in fp32 PSUM.
"""

import numpy as np
import ml_dtypes

N, D, V_VOCAB, T = 16384, 256, 32000, 128
DECAY = 0.97
N_CORES = 8
NS = N // N_CORES           # 2048 rows per core
NT = NS // 128              # 16 tiles of 128
YSCL = 2.0 ** -5            # y -> fp8 scale (ln(E@y) is scale invariant)

_cache = {}

# chunk processing order == load-completion order (c3 on Pool lands first,
# then c0/SP, c2/Pool, c1/SP)
CHUNK_ORDER = [3, 0, 2, 1]


def _build():
    import concourse.bacc as bacc
    import concourse.mybir as mybir
    import concourse.tile as tile

    f32 = mybir.dt.float32
    bf16 = mybir.dt.bfloat16
    f8 = mybir.dt.float8e4
    AF = mybir.ActivationFunctionType
    ALU = mybir.AluOpType

    nc = bacc.Bacc("TRN2", target_bir_lowering=False, debug=False,
                   num_devices=N_CORES)

    i_dxts = nc.dram_tensor("dxts", [128, 2 * NS], bf16, kind="ExternalInput")
    i_dyts = nc.dram_tensor("dyts", [128, 2 * NS], f8, kind="ExternalInput")
    i_ets = nc.dram_tensor("ets", [128, NT * 256], f8, kind="ExternalInput")
    i_consts = nc.dram_tensor("consts", [128, 768], bf16, kind="ExternalInput")

    o_x = nc.dram_tensor("out_x", [128, NT], f32, kind="ExternalOutput")
    o_a = nc.dram_tensor("out_a", [128, 2], bf16, kind="ExternalOutput")
    # combined tail output: cols 0..NT-1 = y (bf16), cols NT..NT+1 = vs
    o_yv = nc.dram_tensor("out_yv", [128, NT + 2], bf16, kind="ExternalOutput")
    o_rho = nc.dram_tensor("out_rho", [256, NS], bf16, kind="ExternalOutput")

    with tile.TileContext(nc) as tc:
        with (
            tc.tile_pool(name="persist", bufs=1) as pp,
            tc.tile_pool(name="rhobuf", bufs=2) as wp,
            tc.tile_pool(name="psBig", bufs=3, space="PSUM") as psBig,
            tc.tile_pool(name="psRc", bufs=4, space="PSUM") as psRc,
            tc.tile_pool(name="psT", bufs=1, space="PSUM") as psT,
            tc.tile_pool(name="dram", bufs=1, space="DRAM") as dram,
        ):
            consts = pp.tile([128, 768], bf16)
            dxts = pp.tile([128, 2 * NS], bf16)
            dyts = pp.tile([128, 2 * NS], f8)
            ets = pp.tile([128, NT * 256], f8)

            def ccols(c):
                return slice(c * 1024, (c + 1) * 1024)

            # SP ring: consts, dxts chunks 0, 1
            nc.sync.dma_start(consts[:], i_consts[:])
            nc.sync.dma_start(dxts[:, ccols(0)], i_dxts[:, ccols(0)])
            nc.sync.dma_start(dxts[:, ccols(1)], i_dxts[:, ccols(1)])
            # Pool ring: dxts chunk 3, chunk 2, then the tail-only operands
            nc.gpsimd.dma_start(dxts[:, ccols(3)], i_dxts[:, ccols(3)])
            nc.gpsimd.dma_start(dxts[:, ccols(2)], i_dxts[:, ccols(2)])
            nc.gpsimd.dma_start(dyts[:], i_dyts[:])
            nc.gpsimd.dma_start(ets[:], i_ets[:])

            vts = consts[:, 0:256]
            vwu = consts[:, 256:512]   # pre-centered, pre-scaled (2^-10)
            vwp = consts[:, 512:768]

            # one PSUM bank for all small tiles:
            # cols 0-15 y, 16-17 a-cols, 18-19 vs, 20 h
            tail_ps = psT.tile([128, 24], f32, tag="tail")

            rcols = pp.tile([128, NT * 128], bf16)
            rt = pp.tile([128, NS], bf16)
            rtacc = pp.tile([128, 4], f32)
            xfcol = pp.tile([128, NT], f32)
            xfb = pp.tile([128, NT], bf16)
            rho_sbs = []
            for dc in range(2):
                rho_sb = wp.tile([128, NS], bf16, tag="rho")
                rho_sbs.append(rho_sb)

            def rc_chunk(c, eng):
                for j in range(4):
                    i = c * 4 + j
                    base = c * 1024 + j * 128
                    rc_ps = psRc.tile([128, 128], f32, tag="rc")
                    for h in range(2):
                        nc.tensor.matmul(
                            rc_ps[:],
                            lhsT=dxts[:, base + h * 512: base + h * 512 + 128],
                            rhs=vts[:, h * 128:(h + 1) * 128],
                            start=(h == 0), stop=(h == 1))
                    dst = rcols[:, i * 128:(i + 1) * 128]
                    if eng[j] == 0:
                        nc.vector.tensor_scalar(dst, rc_ps[:], 0.0, None,
                                                ALU.max, ALU.add,
                                                accum_out=xfcol[:, i:i + 1])
                    else:
                        nc.scalar.activation(dst, rc_ps[:], AF.Relu,
                                             accum_out=xfcol[:, i:i + 1])

            def rt_chunk(c, e):
                rt_ps = psBig.tile([128, 512], f32, tag="big")
                for h in range(2):
                    nc.tensor.matmul(
                        rt_ps[:],
                        lhsT=vts[:, h * 128:(h + 1) * 128],
                        rhs=dxts[:, c * 1024 + h * 512: c * 1024 + h * 512 + 512],
                        start=(h == 0), stop=(h == 1))
                dst = rt[:, c * 512:(c + 1) * 512]
                if e == 0:
                    nc.vector.tensor_scalar(dst, rt_ps[:], 0.0, None,
                                            ALU.max, ALU.add,
                                            accum_out=rtacc[:, c:c + 1])
                else:
                    nc.scalar.activation(dst, rt_ps[:], AF.Relu,
                                         accum_out=rtacc[:, c:c + 1])

            wring = [nc.sync, nc.scalar, nc.gpsimd]

            def rho_chunk(c, copy_eng, rings):
                for dc in range(2):
                    rho_ps = psBig.tile([128, 512], f32, tag="big")
                    nc.tensor.matmul(rho_ps[:],
                                     lhsT=vwp[:, dc * 128:(dc + 1) * 128],
                                     rhs=rt[:, c * 512:(c + 1) * 512],
                                     start=True, stop=True)
                    dst = rho_sbs[dc][:, c * 512:(c + 1) * 512]
                    if copy_eng[dc] == 0:
                        nc.vector.tensor_copy(dst, rho_ps[:])
                    else:
                        nc.scalar.activation(dst, rho_ps[:], AF.Copy)
                    wring[rings[dc]].dma_start(
                        o_rho[dc * 128:(dc + 1) * 128,
                              c * 512:(c + 1) * 512], dst)

            # ---- PE program ----
            c0, c1, c2, c3 = CHUNK_ORDER
            rc_chunk(c0, [0, 1, 0, 1])
            rt_chunk(c0, 0)
            rc_chunk(c1, [0, 1, 0, 1])
            rt_chunk(c1, 1)
            rho_chunk(c0, [0, 1], [2, 2])
            rc_chunk(c2, [0, 1, 0, 1])
            rt_chunk(c2, 0)
            rho_chunk(c1, [0, 1], [0, 1])
            rc_chunk(c3, [0, 1, 0, 1])
            rt_chunk(c3, 1)
            rho_chunk(c2, [0, 1], [2, 2])

            # ---- a-chain: h = R^T x_f ; a_cols = vwu''^T h ----
            nc.vector.tensor_copy(xfb[:], xfcol[:])
            xfs = pp.tile([128, NT], f32)
            nc.vector.tensor_scalar_mul(xfs[:], xfcol[:], 2.0 ** -8)
            for i in range(NT):
                nc.tensor.matmul(tail_ps[:, 20:21],
                                 lhsT=rcols[:, i * 128:(i + 1) * 128],
                                 rhs=xfb[:, i:i + 1],
                                 start=(i == 0), stop=(i == NT - 1))
            h_sb = pp.tile([128, 1], bf16)
            nc.vector.tensor_copy(h_sb[:], tail_ps[:, 20:21])
            for dc in range(2):
                nc.tensor.matmul(tail_ps[:, 16 + dc:17 + dc],
                                 lhsT=vwu[:, dc * 128:(dc + 1) * 128],
                                 rhs=h_sb[:], start=True, stop=True)
            a_sb = pp.tile([128, 2], bf16)
            nc.vector.tensor_copy(a_sb[:], tail_ps[:, 16:18])

            a_in = dram.tile([128, 2], bf16)
            a_out = dram.tile([128, 2], bf16)
            nc.sync.dma_start(a_in[:], a_sb[:])

            # last rho chunk + remaining writes while a_in flies
            rho_chunk(c3, [0, 1], [0, 1])

            nc.sync.dma_start(o_x[:], xfcol[:])

            nc.gpsimd.collective_compute(
                "AllReduce", ALU.add,
                replica_groups=[list(range(N_CORES))],
                ins=[a_in.opt()], outs=[a_out.opt()],
            )

            # ---- tail ----
            afull = pp.tile([128, 2], bf16)
            nc.sync.dma_start(afull[:], a_out[:])
            nc.scalar.dma_start(o_a[:], afull[:])

            aln = pp.tile([128, 2], f8)
            nc.vector.tensor_copy(aln[:], afull[:])

            # y = relu(Dy @ aln)/16 * x_f   (fused relu*xf via stt)
            for i in range(NT):
                for h in range(2):
                    nc.tensor.matmul(
                        tail_ps[:, i:i + 1],
                        lhsT=dyts[:, h * NS + i * 128: h * NS + (i + 1) * 128],
                        rhs=aln[:, h:h + 1],
                        start=(h == 0), stop=(h == 1))
            y_f8 = pp.tile([128, NT], f8)
            yv_sb = pp.tile([128, NT + 2], bf16)
            nc.vector.scalar_tensor_tensor(
                y_f8[:], tail_ps[:, 0:NT], 0.0, xfs[:],
                op0=ALU.max, op1=ALU.mult)
            nc.vector.scalar_tensor_tensor(
                yv_sb[:, 0:NT], tail_ps[:, 0:NT], 0.0, xfb[:],
                op0=ALU.max, op1=ALU.mult)

            # vs partial = E @ y  (E shipped x64; host ln is scale-inv)
            for h in range(2):
                for i in range(NT):
                    nc.tensor.matmul(
                        tail_ps[:, 18 + h:19 + h],
                        lhsT=ets[:, i * 256 + h * 128: i * 256 + (h + 1) * 128],
                        rhs=y_f8[:, i:i + 1],
                        start=(i == 0), stop=(i == NT - 1))
            nc.vector.tensor_copy(yv_sb[:, NT:NT + 2], tail_ps[:, 18:20])
            nc.sync.dma_start(o_yv[:], yv_sb[:])

    nc.finalize()
    return nc


def _host_prep(E, Dx, Dy, token_emb, tokens):
    bf = ml_dtypes.bfloat16
    f8 = ml_dtypes.float8_e4m3fn
    E = np.asarray(E, dtype=np.float32)
    Dx = np.asarray(Dx, dtype=np.float32)
    Dy = np.asarray(Dy, dtype=np.float32)
    token_emb = np.asarray(token_emb, dtype=np.float32)
    tokens = np.asarray(tokens).astype(np.int64)

    v = np.ascontiguousarray(token_emb[tokens])          # [T, d]
    vts = np.concatenate([v[:, :128].T, v[:, 128:].T], axis=1)  # [128, 256]
    j = np.arange(T)
    w = (DECAY ** ((T - 1) - j)).astype(np.float32)
    w[T - 1] = 0.0
    wp = (DECAY ** (T - j)).astype(np.float32)
    u = np.triu(np.ones((T, T), dtype=np.float32))
    vwu = u @ (v * w[:, None])                           # [T, d]
    vwp = u @ (v * wp[:, None])
    # fold the ln centering and the 2^-10 pre-scale into vwu:
    # a_cols = vwu''^T h  comes out centered+scaled on the device.
    vwu = (vwu - vwu.mean(axis=1, keepdims=True)) * 2.0 ** -10
    consts = np.ascontiguousarray(
        np.concatenate([vts, vwu, vwp], axis=1)).astype(bf)

    in_maps = []
    for k in range(N_CORES):
        sl = slice(k * NS, (k + 1) * NS)
        dx_s = Dx[sl]                                    # [NS, 256]
        dy_s = Dy[sl]
        e_s = E[:, sl]                                   # [256, NS]
        dxts = np.empty((128, 2 * NS), np.float32)
        for c in range(4):
            nsl = slice(c * 512, (c + 1) * 512)
            dxts[:, c * 1024: c * 1024 + 512] = dx_s[nsl, :128].T
            dxts[:, c * 1024 + 512: (c + 1) * 1024] = dx_s[nsl, 128:].T
        dyts = np.concatenate([dy_s[:, :128].T, dy_s[:, 128:].T],
                              axis=1) * 16.0
        ets = np.concatenate(
            [e_s[:, i * 128:(i + 1) * 128].T for i in range(NT)],
            axis=1) * 64.0
        in_maps.append({
            "dxts": np.ascontiguousarray(dxts).astype(bf),
            "dyts": np.ascontiguousarray(dyts).astype(f8),
            "ets": np.ascontiguousarray(ets).astype(f8),
            "consts": consts,
        })
    return in_maps


def _ln_host(z, eps=1e-6):
    m = z.mean()
    s = z.std(ddof=1)
    return (z - m) / (s + eps)


def kernel(E, Dx, Dy, token_emb, tokens, _trace=False):
    from concourse.bass_utils import run_bass_kernel_spmd

    key = "nc"
    if key not in _cache:
        _cache[key] = _build()
    nc = _cache[key]

    in_maps = _host_prep(E, Dx, Dy, token_emb, tokens)
    res = run_bass_kernel_spmd(nc, in_maps, core_ids=list(range(N_CORES)),
                               trace=_trace)
    _cache["last_result"] = res

    r = res.results
    x_full = np.concatenate(
        [np.asarray(r[k]["out_x"], np.float32).T.ravel()
         for k in range(N_CORES)])
    # out_a holds (a - mean) * 2^-10 in column layout [128, 2]
    a_cent = np.asarray(r[0]["out_a"], np.float32).T.ravel() * 2.0 ** 10
    yfac = 64.0 / (a_cent.std(ddof=1) + 1e-6)
    y_full = np.concatenate(
        [np.asarray(r[k]["out_yv"]).astype(np.float32)[:, 0:NT].T.ravel()
         * yfac for k in range(N_CORES)])
    vs_raw = np.zeros(256, np.float64)
    for k in range(N_CORES):
        vs_raw += np.asarray(r[k]["out_yv"]).astype(np.float32)[:, NT:].T.ravel()
    vs = _ln_host(vs_raw.astype(np.float32))
    rho = np.concatenate(
        [np.asarray(r[k]["out_rho"]).astype(np.float32)
         for k in range(N_CORES)], axis=1)
    return np.concatenate([x_full, y_full, vs, rho.ravel()]).astype(np.float32)
